# revision 1
# baseline (speedup 1.0000x reference)
"""CrossTransFormer attention kernel for 8x Trainium2 NeuronCores (Bass/Tile).

Problem (per batch b, B=8, C=773, P=4096):
    K = Wk @ Xk + bk            [C, P]
    V = Wv @ Xq + bv            [C, P]
    S[i, j] = sum_c K[c, i] * V[c, j] / sqrt(C)       (i, j over P)
    H = softmax(S, axis=i)
    out[k, j] = sum_i Xk[k, i] * H[i, j]              [C, P]

Sharding: data-parallel over batch, one batch per NeuronCore, no collectives.

Per-core algorithm (all matmul operands fp16, fp32 PSUM accumulation;
softmax without max-subtraction — S ~ N(0,1) so exp is safe in fp32):
  A) Transpose Wk/Wv into [c, o] layout on the PE (fp16), appending the bias
     as an extra contraction row (paired with a ones-row in the activations)
     so the bias add is free inside the matmul.
  B) Stream Xk: K_proj = WkT.T @ Xk kept resident in SBUF fp16 [o, i];
     also transpose Xk blocks on PE into QT = Xk^T fp16 [i, k] resident,
     with an appended ones *column* so the second matmul also computes
     softmax column sums for free.
  C) Stream Xq: V_proj = WvT.T @ Xq staged to internal DRAM fp16.
  D) For each j-block of 512: stream V_proj[:, jblk]; loop i-tiles of 128:
     S-psum = K_proj_tile.T @ V_block (7 matmuls), exp via ACT with the
     1/sqrt(C) scale fused, out_acc[k, j] += QT_tile.T @ expS (7 matmuls,
     held in 7 PSUM banks across the whole i-loop; 8th bank ping-pongs S).
     Epilogue: reciprocal of the sums row, gpsimd partition-broadcast,
     multiply, DMA out.
"""

import sys

sys.path.insert(0, "/opt/trn_rl_repo")

import numpy as np

import concourse.bacc as bacc
import concourse.mybir as mybir
import concourse.tile as tile
from concourse.bass_utils import run_bass_kernel_spmd
from concourse.masks import make_identity

F32 = mybir.dt.float32
F16 = mybir.dt.float16

C = 773
PT = 128
CT = 7  # ceil(773 / 128) chunks of the channel dim
LC = C - (CT - 1) * PT  # 5 rows in the last chunk
JB = 512  # j-block width (one PSUM bank of fp32)


def build(P=4096, n_cores=8):
    NJ = P // JB
    IT = P // PT
    SCALE = float(1.0 / np.sqrt(C))

    nc = bacc.Bacc("TRN2", target_bir_lowering=False, debug=False,
                   num_devices=n_cores)
    Xq = nc.dram_tensor("Xq", [C, P], F32, kind="ExternalInput")
    Xk = nc.dram_tensor("Xk", [C, P], F32, kind="ExternalInput")
    Wk = nc.dram_tensor("Wk", [C, C], F32, kind="ExternalInput")
    bk = nc.dram_tensor("bk", [C], F32, kind="ExternalInput")
    Wv = nc.dram_tensor("Wv", [C, C], F32, kind="ExternalInput")
    bv = nc.dram_tensor("bv", [C], F32, kind="ExternalInput")
    out = nc.dram_tensor("out", [C, P], F32, kind="ExternalOutput")

    with tile.TileContext(nc) as tc:
        with (
            tc.tile_pool(name="persist", bufs=1) as persist,
            tc.tile_pool(name="dram", bufs=1, space="DRAM") as dram,
        ):
            ident = persist.tile([PT, PT], F16)
            make_identity(nc, ident)

            # [c-in-chunk, ct, o]; last chunk rows: 0..4 = W^T rows, 5 = bias
            wkT = persist.tile([PT, CT, C], F16)
            wvT = persist.tile([PT, CT, C], F16)
            # [o-in-tile, ot, i]; last tile rows 5..127 stay zero
            kproj = persist.tile([PT, CT, P], F16)
            # [i-in-tile, it, k]; cols 773..799 zero, col 800 all-ones so the
            # last out-matmul tile (mk=33) lands the softmax sums on PSUM
            # partition 32 (compute-engine SBUF/partition base must be
            # 0/32/64/96)
            qt = persist.tile([PT, IT, C + 28], F16)
            # [o-in-part, ot, j] staged V projection
            vdram = dram.tile([PT, CT, P], F16)
            # persistent fp16 ones row, DMA'd into the bias-trick position
            ones16 = persist.tile([1, JB], F16)
            nc.gpsimd.memset(ones16[:, :], 1.0)

            nc.gpsimd.memset(wkT[:, CT - 1, :], 0.0)
            nc.gpsimd.memset(wvT[:, CT - 1, :], 0.0)
            nc.gpsimd.memset(kproj[:, CT - 1, :], 0.0)
            nc.gpsimd.memset(qt[:, :, C:], 0.0)
            nc.gpsimd.memset(qt[:, :, C + 27:], 1.0)

            # PE warmup: ~5us of dummy matmuls so the HAM clock-gate opens
            # (4/8 -> 8/8) while the first weight DMAs are still in flight,
            # and the exp activation table loads before the main loop.
            warm = persist.tile([PT, JB], F16)
            nc.gpsimd.memset(warm[:, :], 0.0)
            with tc.tile_pool(name="pswarm", bufs=1, space="PSUM") as pswarm:
                wps = pswarm.tile([PT, JB], F32)
                for _ in range(24):
                    nc.tensor.matmul(wps[:, :], warm[:, :PT], warm[:, :],
                                     start=True, stop=True,
                                     skip_group_check=True)
                wexp = persist.tile([1, 16], F32)
                nc.scalar.activation(wexp[:], wps[:1, :16],
                                     mybir.ActivationFunctionType.Exp,
                                     scale=1.0)

            # ---- Phase A: weight transposes (PE) + bias rows ----
            with (
                tc.tile_pool(name="wload", bufs=2) as wload,
                tc.tile_pool(name="pst", bufs=4, space="PSUM") as pst,
            ):
                for Wsrc, bsrc, dstT in ((Wk, bk, wkT), (Wv, bv, wvT)):
                    for ot in range(CT):
                        po = PT if ot < CT - 1 else LC
                        wf = wload.tile([PT, C], F32, tag="wf")
                        nc.sync.dma_start(wf[:po, :],
                                          Wsrc[ot * PT:ot * PT + po, :])
                        w16 = wload.tile([PT, C], F16, tag="w16")
                        nc.vector.tensor_copy(w16[:po, :], wf[:po, :])
                        for ct in range(CT):
                            pc = PT if ct < CT - 1 else LC
                            ps = pst.tile([PT, PT], F16, tag="pst")
                            nc.tensor.transpose(
                                ps[:pc, :po],
                                w16[:po, ct * PT:ct * PT + pc],
                                ident[:po, :po],
                            )
                            nc.any.tensor_copy(
                                dstT[:pc, ct, ot * PT:ot * PT + po],
                                ps[:pc, :po],
                            )
                    bf = wload.tile([1, C], F32, tag="bf")
                    nc.sync.dma_start(bf[:, :], bsrc[None, :])
                    b16 = wload.tile([1, C], F16, tag="b16")
                    nc.vector.tensor_copy(b16[:], bf[:])
                    nc.sync.dma_start(dstT[LC:LC + 1, CT - 1, :], b16[:, :])

            # ---- Phases B & C: projections (+ QT transposes) ----
            with (
                tc.tile_pool(name="xstream", bufs=2) as xs,
                tc.tile_pool(name="psk", bufs=2, space="PSUM") as psk,
                tc.tile_pool(name="pst2", bufs=4, space="PSUM") as pst2,
            ):
                def stream_chunk(src, jc):
                    """DMA a [C, 512] chunk, append ones row, zero pad, cast."""
                    xf = xs.tile([PT, CT, JB], F32, tag="xf")
                    nc.gpsimd.memset(xf[:, CT - 1, :], 0.0)
                    for ct in range(CT):
                        pc = PT if ct < CT - 1 else LC
                        nc.sync.dma_start(
                            xf[:pc, ct, :],
                            src[ct * PT:ct * PT + pc, jc * JB:(jc + 1) * JB],
                        )
                    x16 = xs.tile([PT, CT, JB], F16, tag="x16")
                    nc.vector.tensor_copy(x16[:], xf[:])
                    # bias-trick ones row (pairs with the bias row in w*T)
                    nc.sync.dma_start(x16[LC:LC + 1, CT - 1, :], ones16[:, :])
                    return x16

                def project(wT, x16, ot):
                    po = PT if ot < CT - 1 else LC
                    ps = psk.tile([PT, JB], F32, tag="psk")
                    for ct in range(CT):
                        nc.tensor.matmul(
                            ps[:po, :],
                            wT[:, ct, ot * PT:ot * PT + po],
                            x16[:, ct, :],
                            start=(ct == 0),
                            stop=(ct == CT - 1),
                        )
                    return ps, po

                # Phase B: Xk -> kproj (resident) and qt (PE transposes)
                for jc in range(NJ):
                    x16 = stream_chunk(Xk, jc)
                    for ot in range(CT):
                        ps, po = project(wkT, x16, ot)
                        nc.any.tensor_copy(
                            kproj[:po, ot, jc * JB:(jc + 1) * JB], ps[:po, :]
                        )
                    for sub in range(JB // PT):
                        it = jc * (JB // PT) + sub
                        for kt in range(CT):
                            pk = PT if kt < CT - 1 else LC
                            ps = pst2.tile([PT, PT], F16, tag="pst2")
                            nc.tensor.transpose(
                                ps[:, :pk],
                                x16[:pk, kt, sub * PT:(sub + 1) * PT],
                                ident[:pk, :pk],
                            )
                            nc.any.tensor_copy(
                                qt[:, it, kt * PT:kt * PT + pk], ps[:, :pk]
                            )

                # Phase C: Xq -> vdram
                z16 = xs.tile([PT, JB], F16, tag="z16")
                nc.gpsimd.memset(z16[:, :], 0.0)
                for jc in range(NJ):
                    x16 = stream_chunk(Xq, jc)
                    for ot in range(CT):
                        ps, po = project(wvT, x16, ot)
                        v16 = xs.tile([PT, JB], F16, tag="v16")
                        nc.any.tensor_copy(v16[:po, :], ps[:po, :])
                        nc.sync.dma_start(
                            vdram[:po, ot, jc * JB:(jc + 1) * JB], v16[:po, :]
                        )
                    # rows LC..127 of the last o-tile must be zero on reload
                    nc.sync.dma_start(
                        vdram[LC:, CT - 1, jc * JB:(jc + 1) * JB],
                        z16[:PT - LC, :],
                    )

            # ---- Phase D: attention main loop ----
            with (
                tc.tile_pool(name="vp", bufs=2) as vp,
                tc.tile_pool(name="ep", bufs=3) as epl,
                tc.tile_pool(name="op", bufs=2) as op,
                tc.tile_pool(name="rp", bufs=2) as rp,
                tc.tile_pool(name="psacc", bufs=CT, space="PSUM") as psacc,
                tc.tile_pool(name="pss", bufs=1, space="PSUM") as pss,
            ):
                def fetch_v(jb):
                    vst = vp.tile([PT, CT, JB], F16, tag="vst",
                                  name=f"vst{jb}")
                    nc.sync.dma_start(vst[:],
                                      vdram[:, :, jb * JB:(jb + 1) * JB])
                    return vst

                vst_next = fetch_v(0)
                for jb in range(NJ):
                    js = slice(jb * JB, (jb + 1) * JB)
                    vst = vst_next

                    acc = [psacc.tile([PT, JB], F32, tag="acc", name=f"acc{jb}_{i}")
                           for i in range(CT)]

                    def emit_S(t):
                        ps_s = pss.tile([PT, JB], F32, tag="s")
                        for ct in range(CT):
                            nc.tensor.matmul(
                                ps_s[:, :],
                                kproj[:, ct, t * PT:(t + 1) * PT],
                                vst[:, ct, :],
                                start=(ct == 0),
                                stop=(ct == CT - 1),
                                skip_group_check=True,
                            )
                        return ps_s

                    # software-pipelined: S(t+1) is emitted before out(t) so
                    # the PE never waits on the ACT exp
                    ps_cur = emit_S(0)
                    for t in range(IT):
                        ps_next = emit_S(t + 1) if t < IT - 1 else None
                        es = epl.tile([PT, JB], F16, tag="es")
                        nc.scalar.activation(
                            es[:], ps_cur[:],
                            mybir.ActivationFunctionType.Exp, scale=SCALE,
                        )
                        for kt in range(CT):
                            mk = PT if kt < CT - 1 else 33
                            nc.tensor.matmul(
                                acc[kt][:mk, :],
                                qt[:, t, kt * PT:kt * PT + mk],
                                es[:],
                                start=(t == 0),
                                stop=(t == IT - 1),
                                skip_group_check=True,
                            )
                        ps_cur = ps_next

                    # prefetch next block's V before the epilogue so its DMA
                    # isn't FIFO-blocked behind the output store on the ring
                    if jb < NJ - 1:
                        vst_next = fetch_v(jb + 1)

                    recip = rp.tile([1, JB], F32, tag="recip")
                    nc.vector.reciprocal(recip[:], acc[CT - 1][32:33, :])
                    bc = rp.tile([PT, JB], F32, tag="bc")
                    nc.gpsimd.partition_broadcast(bc[:], recip[:])
                    osb = op.tile([PT, CT, JB], F32, tag="osb")
                    for kt in range(CT):
                        mk = PT if kt < CT - 1 else LC
                        nc.vector.tensor_mul(
                            out=osb[:mk, kt, :], in0=acc[kt][:mk, :],
                            in1=bc[:mk, :],
                        )
                    # vst prefetch is emitted above, so these stores queue
                    # behind it (not in front of it) on the ring
                    nc.sync.dma_start(
                        out[0:(CT - 1) * PT, js].rearrange(
                            "(kt p) j -> p kt j", p=PT),
                        osb[:, 0:CT - 1, :],
                    )
                    nc.sync.dma_start(out[(CT - 1) * PT:C, js],
                                      osb[:LC, CT - 1, :])

    nc.compile()
    return nc


_CACHE = {}


def _get_program(P=4096, n_cores=8):
    key = (P, n_cores)
    if key not in _CACHE:
        _CACHE[key] = build(P, n_cores)
    return _CACHE[key]


def _run(inputs, trace=False, **kw):
    nc = _get_program()
    Xq = np.asarray(inputs["Xq"], dtype=np.float32)
    Xk = np.asarray(inputs["Xk"], dtype=np.float32)
    Wk = np.ascontiguousarray(np.asarray(inputs["Wk"], dtype=np.float32))
    bkv = np.ascontiguousarray(np.asarray(inputs["bk"], dtype=np.float32))
    Wv = np.ascontiguousarray(np.asarray(inputs["Wv"], dtype=np.float32))
    bvv = np.ascontiguousarray(np.asarray(inputs["bv"], dtype=np.float32))
    B = Xq.shape[0]
    in_maps = [
        {
            "Xq": np.ascontiguousarray(Xq[b]),
            "Xk": np.ascontiguousarray(Xk[b]),
            "Wk": Wk,
            "bk": bkv,
            "Wv": Wv,
            "bv": bvv,
        }
        for b in range(B)
    ]
    res = run_bass_kernel_spmd(nc, in_maps, list(range(B)), trace=trace, **kw)
    outs = np.stack([res.results[b]["out"] for b in range(B)], axis=0)
    return outs.astype(np.float32), res


def kernel(**inputs):
    outs, _ = _run(inputs)
    return outs

